# revision 1
# baseline (speedup 1.0000x reference)
"""GQA attention block (RoPE + causal softmax + out-proj) on 8 TRN2 cores.

Sharding: 8 cores = 2 batches x 4 kv-pairs. Core c handles batch c//4 and
kv heads {2p, 2p+1} (p = c%4), i.e. q heads 6p..6p+5. Each core computes its
partial y^T = wo_slice^T @ attn_out^T; the host sums the 4 partials per batch
and transposes back.

Per-core layout: everything stays feature-major [d, s] so no on-device
transposes of large activations are needed:
  Q^T/K^T: [128d, s]   (projection emits them directly)
  scores come out transposed: [t, s] blocks from lhsT=K^T-slice, rhs=Q^T
  probs [t, s] feed AV directly with V in [t, dv] (via small PE transposes)
RoPE is applied in [d, s] form by permuting the head dim on the HOST to
[evens | odds]; the rotation becomes a partition-block swap (done with a PE
permutation matmul) plus elementwise mul/adds. The softmax scale is folded
into wq on the host. Softmax runs without max-subtraction (scores are O(10),
exp is safe in fp32): probs = exp(scores) * binary_causal_mask, row sums via
an all-ones matmul into PSUM, and 1/l is applied to the AV output after a PE
broadcast of l and a fast DVE reciprocal.

Weights / activations are pre-tiled on the host into the exact SBUF tile
layouts so every streaming DMA is one big contiguous descriptor.
"""

import math
from contextlib import ExitStack

import numpy as np
import ml_dtypes

import concourse.bass as bass
import concourse.mybir as mybir
import concourse.tile as tile
from concourse import bacc
from concourse.bass_utils import run_bass_kernel_spmd
from concourse.masks import make_identity

B, S, DIM = 2, 2048, 3072
NH, NKV, HD = 24, 8, 128
QT_PER_CORE = 6   # q head-tiles per core
KV_PER_CORE = 2   # kv heads per core
NDT = QT_PER_CORE + 2 * KV_PER_CORE  # 10 projection d-tiles
NKT = DIM // 128  # 24 contraction tiles
SW = 512          # s-window (matmul moving free dim)
NJ = S // SW      # 4 windows
NTT = S // 128    # 16 t-tiles
SCALE = 1.0 / math.sqrt(HD)

F32 = mybir.dt.float32
F32R = mybir.dt.float32r
BF16 = mybir.dt.bfloat16
BF = ml_dtypes.bfloat16

_PERM = np.concatenate([np.arange(0, HD, 2), np.arange(1, HD, 2)])


def _build_body(nc, tc, io, ctx):
    x4, w10, wo4 = io["x4"], io["w10"], io["wo4"]
    ropeC, ropeS, masks, swp, yT = (
        io["ropeC"], io["ropeS"], io["masks"], io["swp"], io["yT"])

    singles = ctx.enter_context(tc.tile_pool(name="singles", bufs=1))
    ps = ctx.enter_context(tc.tile_pool(name="ps", bufs=1, space=bass.MemorySpace.PSUM))
    xt_pool = ctx.enter_context(tc.tile_pool(name="xtp", bufs=2))
    w_pool = ctx.enter_context(tc.tile_pool(name="wtp", bufs=5))
    wo_pool = ctx.enter_context(tc.tile_pool(name="wotp", bufs=6))
    raw_pool = ctx.enter_context(tc.tile_pool(name="rawp", bufs=3))
    qT_pool = ctx.enter_context(tc.tile_pool(name="qTp", bufs=7))
    probs_pool = ctx.enter_context(tc.tile_pool(name="prp", bufs=8))
    small_pool = ctx.enter_context(tc.tile_pool(name="smp", bufs=2))
    out_pool = ctx.enter_context(tc.tile_pool(name="otp", bufs=7))
    y_pool = ctx.enter_context(tc.tile_pool(name="yp", bufs=3))

    # constants / persistent state (const DMAs ride the gpsimd queue so they
    # don't delay the first x/weight loads on sync)
    ropeC_sb = singles.tile([128, S], F32, tag="ropeC", name="ropeC_sb")
    ropeS_sb = singles.tile([128, S], F32, tag="ropeS", name="ropeS_sb")
    masks_sb = singles.tile([128, 4, SW], BF16, tag="masks", name="masks_sb")
    swp_sb = singles.tile([128, 128], BF16, tag="swp", name="swp_sb")
    ident = singles.tile([128, 128], F32, tag="ident", name="ident")
    ones_t = singles.tile([128, 1], BF16, tag="ones_t", name="ones_t")
    ones_mat = singles.tile([128, 128], F32R, tag="ones_mat", name="ones_mat")
    ones_mat0 = singles.tile([128, 128], F32, tag="ones_mat0", name="ones_mat0")
    nc.gpsimd.dma_start(out=ropeC_sb, in_=ropeC[:])
    nc.gpsimd.dma_start(out=ropeS_sb, in_=ropeS[:])
    nc.gpsimd.dma_start(out=masks_sb, in_=masks[:])
    nc.gpsimd.dma_start(out=swp_sb, in_=swp[:])
    make_identity(nc, ident)
    nc.vector.memset(ones_t, 1.0)
    nc.vector.memset(ones_mat0, 1.0)
    nc.scalar.copy(out=ones_mat, in_=ones_mat0)

    KT_sb = [singles.tile([128, S], BF16, tag=f"KT{g}", name=f"KT{g}")
             for g in range(KV_PER_CORE)]
    V_sb = [singles.tile([128, NTT, 128], BF16, tag=f"V{g}", name=f"V{g}")
            for g in range(KV_PER_CORE)]

    for j in range(NJ):
        jw = bass.ts(j, SW)

        # first weight tile ahead of the x slices so the PE can start early;
        # split across both HWDGE queues (one queue moves ~65GB/s)
        wt0 = w_pool.tile([128, NKT, 128], BF16, name="wt")
        nc.sync.dma_start(out=wt0[:, 0:12, :], in_=w10[0, :, 0:12, :])
        nc.scalar.dma_start(out=wt0[:, 12:24, :], in_=w10[0, :, 12:24, :])
        # x chunk: fine slices up front so the first matmuls start early,
        # coarser after, alternating queues
        xt = xt_pool.tile([128, NKT, SW], BF16, name="xt")
        xsl = [(0, 1), (1, 2), (2, 4), (4, 6), (6, 9), (9, 12),
               (12, 16), (16, 20), (20, 24)]
        for n, (k0, k1) in enumerate(xsl):
            eng = nc.sync if n % 2 == 0 else nc.scalar
            eng.dma_start(out=xt[:, k0:k1, :], in_=x4[j, :, k0:k1, :])

        # ---- projections for window j ----
        # dt 0..5 -> q tiles, 6..7 -> k heads, 8..9 -> v heads.
        # Post-processing (swap matmul / V transposes + rope) for tile dt is
        # deferred until after tile dt+1's accumulation so the PE never waits
        # on the ACT evacuation of PSUM.
        qTj = [None] * QT_PER_CORE
        post = []

        def _post_q(dt, raw, sw_ps):
            def run():
                if dt < 6:
                    qt = qT_pool.tile([128, SW], BF16, name="qt")
                    qTj[dt] = qt
                    dest = qt
                else:
                    dest = KT_sb[dt - 6][:, jw]
                nc.vector.tensor_mul(dest, raw, ropeC_sb[:, jw])
                t2 = raw_pool.tile([128, SW], BF16, name="t2")
                nc.vector.tensor_mul(t2, sw_ps, ropeS_sb[:, jw])
                nc.vector.tensor_add(dest, dest, t2)
            return run

        def _post_v(dt, vraw):
            def run():
                g = dt - 8
                tp = ps.tile([128, SW], F32, tag="pp", bufs=2, name="tp")
                for rr in range(4):
                    nc.tensor.transpose(tp[:, bass.ts(rr, 128)],
                                        vraw[:, bass.ts(rr, 128)], ident)
                nc.scalar.copy(out=V_sb[g][:, 4 * j:4 * j + 4, :],
                               in_=tp.rearrange("p (r t) -> p r t", r=4))
            return run

        for dt in range(NDT):
            kind = "q" if dt < 6 else ("k" if dt < 8 else "v")
            if dt == 0:
                wt = wt0
            else:
                wt = w_pool.tile([128, NKT, 128], BF16, name="wt")
                nc.sync.dma_start(out=wt, in_=w10[dt])
            pp = ps.tile([128, SW], F32, tag="pp", bufs=2, name="pp")
            for k in range(NKT):
                nc.tensor.matmul(pp, wt[:, k, :], xt[:, k, :],
                                 start=(k == 0), stop=(k == NKT - 1))
            if kind == "v":
                vraw = raw_pool.tile([128, SW], F32, name="vraw")
                nc.any.tensor_copy(out=vraw, in_=pp)
                post.append(_post_v(dt, vraw))
            else:
                raw = raw_pool.tile([128, SW], BF16, name="raw")
                nc.any.tensor_copy(out=raw, in_=pp)
                sw_ps = ps.tile([128, SW], F32, tag="pp", bufs=2, name="sw_ps")
                nc.tensor.matmul(sw_ps, swp_sb, raw, start=True, stop=True)
                post.append(_post_q(dt, raw, sw_ps))
            if len(post) > 1:
                post.pop(0)()
        post.pop(0)()

        # ---- attention for window j ----
        # Units (q head-tiles) run pairwise-interleaved so the ACT exp chain
        # of one unit hides behind the other unit's matmuls. The two l
        # accumulators share one PSUM bank at 32-aligned partition rows.
        outTj = [None] * QT_PER_CORE
        nlast = 4 * j + 3
        for pair in range(QT_PER_CORE // 2):
            units = (2 * pair, 2 * pair + 1)
            lpack = ps.tile([128, SW], F32, tag="l", bufs=1, name="lpack")
            l_sbp = small_pool.tile([64, SW], F32R, tag="l_sbp", name="l_sbp")
            avs = []
            for a in range(2):
                av = ps.tile([128, SW], F32, tag="av", bufs=2, name="av")
                avs.append(av)
            for i in range(4 * j + 4):
                for a in range(2):
                    u = units[a]
                    g = u // 3
                    sc = ps.tile([128, SW], F32, tag="sy", bufs=3, name="sc")
                    nc.tensor.matmul(sc, KT_sb[g][:, bass.ts(i, 128)], qTj[u],
                                     start=True, stop=True)
                    pr = probs_pool.tile([128, SW], BF16, name="pr")
                    nc.scalar.activation(out=pr, in_=sc,
                                         func=mybir.ActivationFunctionType.Exp)
                    if i >= 4 * j:
                        nc.vector.tensor_mul(pr, pr, masks_sb[:, i - 4 * j, :])
                    nc.tensor.matmul(lpack[32 * a:32 * a + 1, :], ones_t, pr,
                                     start=(i == 0), stop=(i == nlast),
                                     skip_group_check=True)
                    nc.tensor.matmul(avs[a], V_sb[g][:, i, :], pr,
                                     start=(i == 0), stop=(i == nlast),
                                     skip_group_check=True)
            for a in range(2):
                u = units[a]
                row = slice(32 * a, 32 * a + 1)
                nc.scalar.copy(out=l_sbp[row, :], in_=lpack[row, :])
                rb = ps.tile([128, SW], F32, tag="sy", bufs=3, name="rb")
                nc.tensor.matmul(rb, ones_mat[row, :], l_sbp[row, :],
                                 start=True, stop=True)
                rbs = small_pool.tile([128, SW], F32, tag="rbs", name="rbs")
                nc.vector.reciprocal_approx_fast(out=rbs, in_=rb)
                ot = out_pool.tile([128, SW], BF16, name="ot")
                nc.vector.tensor_mul(ot, avs[a], rbs)
                outTj[u] = ot

        # ---- out-projection for window j ----
        for d in range(NKT):
            wot = wo_pool.tile([128, QT_PER_CORE, 128], BF16, name="wot")
            nc.scalar.dma_start(out=wot, in_=wo4[d])
            yp = ps.tile([128, SW], F32, tag="sy", bufs=3, name="yp")
            for u in range(QT_PER_CORE):
                nc.tensor.matmul(yp, wot[:, u, :], outTj[u],
                                 start=(u == 0), stop=(u == QT_PER_CORE - 1))
            ys = y_pool.tile([128, SW], F32, name="ys")
            nc.vector.tensor_copy(out=ys, in_=yp)
            nc.sync.dma_start(out=yT[bass.ts(d, 128), jw], in_=ys)


def build_nc():
    nc = bacc.Bacc("TRN2", target_bir_lowering=False, debug=False, num_devices=8)
    io = {
        "x4": nc.dram_tensor("x4", [NJ, 128, NKT, SW], BF16, kind="ExternalInput"),
        "w10": nc.dram_tensor("w10", [NDT, 128, NKT, 128], BF16, kind="ExternalInput"),
        "wo4": nc.dram_tensor("wo4", [NKT, 128, QT_PER_CORE, 128], BF16,
                              kind="ExternalInput"),
        "ropeC": nc.dram_tensor("ropeC", [HD, S], F32, kind="ExternalInput"),
        "ropeS": nc.dram_tensor("ropeS", [HD, S], F32, kind="ExternalInput"),
        "masks": nc.dram_tensor("masks", [128, 4, SW], BF16, kind="ExternalInput"),
        "swp": nc.dram_tensor("swp", [128, 128], BF16, kind="ExternalInput"),
        "yT": nc.dram_tensor("yT", [DIM, S], F32, kind="ExternalOutput"),
    }
    with tile.TileContext(nc) as tc:
        with ExitStack() as ctx:
            _build_body(nc, tc, io, ctx)
    nc.compile()
    return nc


_NC = None


def _get_nc():
    global _NC
    if _NC is None:
        _NC = build_nc()
    return _NC


def make_in_maps(x, wq, wk, wv, wo, freqs_cos, freqs_sin):
    x = np.asarray(x, np.float32)
    wq = np.asarray(wq, np.float32)
    wk = np.asarray(wk, np.float32)
    wv = np.asarray(wv, np.float32)
    wo = np.asarray(wo, np.float32)
    cos = np.asarray(freqs_cos, np.float32)
    sin = np.asarray(freqs_sin, np.float32)

    wq_p = (wq.reshape(DIM, NH, HD)[:, :, _PERM] * SCALE).astype(BF)
    wk_p = wk.reshape(DIM, NKV, HD)[:, :, _PERM].astype(BF)
    wv_r = wv.reshape(DIM, NKV, HD).astype(BF)
    wo_r = wo.reshape(NH, HD, DIM)

    ropeC = np.ascontiguousarray(np.concatenate([cos.T, cos.T], 0))
    ropeS = np.ascontiguousarray(np.concatenate([-sin.T, sin.T], 0))

    tt = np.arange(128)[:, None]
    ss = np.arange(SW)[None, :]
    # [128, 4, SW] with masks[:, r, :] the r-th diagonal-block pattern
    masks = np.stack([(128 * r + tt <= ss) for r in range(4)], axis=1).astype(BF)

    swp = np.zeros((128, 128), BF)
    swp[np.arange(128), (np.arange(128) + 64) % 128] = 1.0

    in_maps = []
    for c in range(8):
        b, p = divmod(c, 4)
        # per-core weight slices in on-chip tile layout
        wq_c = wq_p[:, 6 * p:6 * p + 6, :]          # [DIM, 6, 128]
        wk_c = wk_p[:, 2 * p:2 * p + 2, :]          # [DIM, 2, 128]
        wv_c = wv_r[:, 2 * p:2 * p + 2, :]          # [DIM, 2, 128]
        # w10[dt] = [128p, 24k, 128d] with DIM rows split as (k, p)
        wcat = np.concatenate([wq_c, wk_c, wv_c], axis=1)   # [DIM, 10, 128]
        w10 = np.ascontiguousarray(
            wcat.reshape(NKT, 128, NDT, HD).transpose(2, 1, 0, 3))
        # wo4[d] = [128p(dv), 6u, 128dd]; wo rows are (u, p)
        wo_c = wo_r[6 * p:6 * p + 6]                 # [6, 128, DIM]
        wo4 = np.ascontiguousarray(
            wo_c.reshape(QT_PER_CORE, HD, NKT, 128).transpose(2, 1, 0, 3)).astype(BF)
        # x4[j] = [128p, 24k, 512s]
        xT_b = x[b].T                                 # [DIM, S]
        x4 = np.ascontiguousarray(
            xT_b.reshape(NKT, 128, NJ, SW).transpose(2, 1, 0, 3)).astype(BF)
        in_maps.append({
            "x4": x4,
            "w10": w10,
            "wo4": wo4,
            "ropeC": ropeC,
            "ropeS": ropeS,
            "masks": masks,
            "swp": swp,
        })
    return in_maps


def gather(results):
    y = np.empty((B, S, DIM), np.float32)
    for b in range(B):
        acc = results[4 * b]["yT"].astype(np.float32)
        for p in range(1, 4):
            acc = acc + results[4 * b + p]["yT"]
        y[b] = acc.T
    return y


def kernel(x, wq, wk, wv, wo, freqs_cos, freqs_sin, **run_kwargs):
    nc = _get_nc()
    in_maps = make_in_maps(x, wq, wk, wv, wo, freqs_cos, freqs_sin)
    res = run_bass_kernel_spmd(nc, in_maps, core_ids=list(range(8)), **run_kwargs)
    out = gather(res.results)
    if run_kwargs:
        return out, res
    return out



# revision 4
# speedup vs baseline: 1.1401x; 1.1401x over previous
"""GQA attention block (RoPE + causal softmax + out-proj) on 8 TRN2 cores.

Sharding: 8 cores = 2 batches x 4 kv-pairs. Core c handles batch c//4 and
kv heads {2p, 2p+1} (p = c%4), i.e. q heads 6p..6p+5. Each core computes its
partial y^T = wo_slice^T @ attn_out^T; the host sums the 4 partials per batch
(bf16 partials, fp32 host accumulation) and transposes back.

Per-core layout: everything stays feature-major [d, s] so no on-device
transposes of large activations are needed. v2 structure:
  - wq/wk/wv tiles persist in SBUF (loaded once, window 0 streams them in).
  - Projections accumulate two 512-col d-tiles into one 2-bank PSUM tile
    so evacuation copies are batched.
  - Attention runs q-head pairs with a depth-2 software pipeline: both
    units' score tiles land in one [128,1024] PSUM tile, ONE batched exp
    serves both, the two l row-sum matmuls issue back-to-back into
    different PE column groups (concurrent), and diagonal blocks are
    causally trimmed to N=512-128r.
  - Out-projection accumulates d-tile pairs and stages bf16 output in
    6-tile groups so each window needs only 4 output DMA descriptors.
Softmax runs without max-subtraction (scores are O(10), exp is safe in
fp32): probs = exp(scores) * binary causal mask, row sums via col-tiled
ones matmuls into PSUM, 1/l applied after a PE broadcast + DVE reciprocal.
"""

import math
from contextlib import ExitStack

import numpy as np
import ml_dtypes

import concourse.bass as bass
import concourse.mybir as mybir
import concourse.tile as tile
from concourse import bacc
from concourse.bass_utils import run_bass_kernel_spmd
from concourse.masks import make_identity

B, S, DIM = 2, 2048, 3072
NH, NKV, HD = 24, 8, 128
QT_PER_CORE = 6   # q head-tiles per core
KV_PER_CORE = 2   # kv heads per core
NDT = QT_PER_CORE + 2 * KV_PER_CORE  # 10 projection d-tiles
NKT = DIM // 128  # 24 contraction tiles
SW = 512          # s-window (matmul moving free dim)
NJ = S // SW      # 4 windows
NTT = S // 128    # 16 t-tiles
NPAIR = NDT // 2  # 5 projection d-tile pairs
SCALE = 1.0 / math.sqrt(HD)

F32 = mybir.dt.float32
F32R = mybir.dt.float32r
BF16 = mybir.dt.bfloat16
BF = ml_dtypes.bfloat16

_PERM = np.concatenate([np.arange(0, HD, 2), np.arange(1, HD, 2)])


def _build_body(nc, tc, io, ctx):
    x4, w10, wo2 = io["x4"], io["w10"], io["wo2"]
    ropeC, ropeS, masks, swp, y4 = (
        io["ropeC"], io["ropeS"], io["masks"], io["swp"], io["y4"])

    singles = ctx.enter_context(tc.tile_pool(name="singles", bufs=1))
    ps = ctx.enter_context(tc.tile_pool(name="ps", bufs=1, space=bass.MemorySpace.PSUM))
    xt_pool = ctx.enter_context(tc.tile_pool(name="xtp", bufs=2))
    wo_pool = ctx.enter_context(tc.tile_pool(name="wotp", bufs=3))
    raw_pool = ctx.enter_context(tc.tile_pool(name="rawp", bufs=2))
    qT_pool = ctx.enter_context(tc.tile_pool(name="qTp", bufs=4))
    probs_pool = ctx.enter_context(tc.tile_pool(name="prp", bufs=4))
    small_pool = ctx.enter_context(tc.tile_pool(name="smp", bufs=2))
    out_pool = ctx.enter_context(tc.tile_pool(name="otp", bufs=7))
    ys_pool = ctx.enter_context(tc.tile_pool(name="ysp", bufs=2))

    # constants / persistent state (const DMAs ride the gpsimd queue so they
    # don't delay the first x/weight loads on sync)
    ropeC_sb = singles.tile([128, S], BF16, tag="ropeC", name="ropeC_sb")
    ropeS_sb = singles.tile([128, S], BF16, tag="ropeS", name="ropeS_sb")
    masks_sb = singles.tile([128, 4, 2 * SW], BF16, tag="masks", name="masks_sb")
    swp_sb = singles.tile([128, 128], BF16, tag="swp", name="swp_sb")
    ident = singles.tile([128, 128], F32, tag="ident", name="ident")
    ones_t = singles.tile([128, 1], BF16, tag="ones_t", name="ones_t")
    ones_mat = singles.tile([128, 128], F32R, tag="ones_mat", name="ones_mat")
    ones_mat0 = singles.tile([128, 128], F32, tag="ones_mat0", name="ones_mat0")
    nc.gpsimd.dma_start(out=ropeC_sb, in_=ropeC[:])
    nc.gpsimd.dma_start(out=ropeS_sb, in_=ropeS[:])
    nc.gpsimd.dma_start(out=masks_sb, in_=masks[:])
    nc.gpsimd.dma_start(out=swp_sb, in_=swp[:])
    make_identity(nc, ident)
    nc.vector.memset(ones_t, 1.0)
    nc.vector.memset(ones_mat0, 1.0)
    nc.scalar.copy(out=ones_mat, in_=ones_mat0)

    # persistent weight tiles: w10_sb[dt] = [128p, 24k, 128d], streamed in by
    # window 0 (fine pair-slices across the gpsimd+vector queues), reused by
    # windows 1-3.
    w10_sb = [singles.tile([128, NKT, 128], BF16, tag=f"w{dt}", name=f"w10_{dt}")
              for dt in range(NDT)]
    KT_sb = [singles.tile([128, S], BF16, tag=f"KT{g}", name=f"KT{g}")
             for g in range(KV_PER_CORE)]
    V_sb = [singles.tile([128, NTT, 128], BF16, tag=f"V{g}", name=f"V{g}")
            for g in range(KV_PER_CORE)]

    # weight DMAs: window-0 JIT stream on the gpsimd queue (each queue sprays
    # across all 16 DMA engines, ~350GB/s aggregate)
    for dt in range(NDT):
        nc.gpsimd.dma_start(out=w10_sb[dt], in_=w10[dt])

    for j in range(NJ):
        jw = bass.ts(j, SW)

        # x chunk: fine slices up front so the first matmuls start early,
        # alternating the sync/scalar queues
        xt = xt_pool.tile([128, NKT, SW], BF16, name="xt")
        xsl = [(0, 1), (1, 2), (2, 4), (4, 6), (6, 9), (9, 12),
               (12, 16), (16, 20), (20, 24)]
        for n, (k0, k1) in enumerate(xsl):
            eng = nc.sync if n % 2 == 0 else nc.scalar
            eng.dma_start(out=xt[:, k0:k1, :], in_=x4[j, :, k0:k1, :])

        # ---- projections for window j (5 d-tile pairs) ----
        # pairs 0-2 -> q tiles, 3 -> k heads, 4 -> v heads. Post-processing
        # (swap matmul / V transposes + rope) for pair p is deferred until
        # after pair p+1's accumulation so the PE never waits on the PSUM
        # evacuation.
        qTj = [None] * (QT_PER_CORE // 2)  # q pair tiles [128, 1024]
        post = []

        def _post_qk(p, raw, sw_ps):
            def run():
                for h in range(2):
                    dt = 2 * p + h
                    hw = bass.ts(h, SW)
                    if dt < 6:
                        if h == 0:
                            qTj[p] = qT_pool.tile([128, 2 * SW], BF16, name="qt")
                        dest = qTj[p][:, hw]
                    else:
                        dest = KT_sb[dt - 6][:, jw]
                    nc.vector.tensor_mul(dest, raw[:, hw], ropeC_sb[:, jw])
                    t2 = raw_pool.tile([128, SW], BF16, tag="t2", bufs=2, name="t2")
                    nc.vector.tensor_mul(t2, sw_ps[:, hw], ropeS_sb[:, jw])
                    nc.vector.tensor_add(dest, dest, t2)
            return run

        def _post_v(vraw):
            def run():
                tp = ps.tile([128, 2 * SW], F32, tag="av", bufs=1, name="tp")
                for g in range(2):
                    for rr in range(4):
                        c = g * 4 + rr
                        nc.tensor.transpose(tp[:, bass.ts(c, 128)],
                                            vraw[:, bass.ts(c, 128)], ident)
                for g in range(2):
                    nc.scalar.copy(
                        out=V_sb[g][:, 4 * j:4 * j + 4, :],
                        in_=tp[:, bass.ts(g, SW)].rearrange("p (r t) -> p r t", r=4))
            return run

        for p in range(NPAIR):
            kind = "q" if p < 3 else ("k" if p == 3 else "v")
            pp = ps.tile([128, 2 * SW], F32, tag="big", bufs=2, name="pp")
            for h in range(2):
                dt = 2 * p + h
                hw = bass.ts(h, SW)
                for k in range(NKT):
                    nc.tensor.matmul(pp[:, hw], w10_sb[dt][:, k, :], xt[:, k, :],
                                     start=(k == 0), stop=(k == NKT - 1))
            if kind == "v":
                vraw = raw_pool.tile([128, 2 * SW], F32, tag="raw", name="vraw")
                nc.any.tensor_copy(out=vraw, in_=pp)
                post.append(_post_v(vraw))
            else:
                raw = raw_pool.tile([128, 2 * SW], BF16, tag="raw", name="raw")
                nc.any.tensor_copy(out=raw, in_=pp)
                sw_ps = ps.tile([128, 2 * SW], F32, tag="av", bufs=1, name="sw_ps")
                for h in range(2):
                    hw = bass.ts(h, SW)
                    nc.tensor.matmul(sw_ps[:, hw], swp_sb, raw[:, hw],
                                     start=True, stop=True)
                post.append(_post_qk(p, raw, sw_ps))
            if len(post) > 1:
                post.pop(0)()
        post.pop(0)()

        # ---- attention for window j: 3 unit-pairs, depth-2 pipeline ----
        outTj = [None] * QT_PER_CORE
        nlast = 4 * j + 3
        for P in range(3):
            u0, u1 = 2 * P, 2 * P + 1
            g0, g1 = u0 // 3, u1 // 3
            qtp = qTj[P]
            lpack = ps.tile([128, SW], F32, tag="aux", bufs=2, name="lpack")
            av = ps.tile([128, 2 * SW], F32, tag="av", bufs=1, name="av")

            def emit_sc(i):
                r = i - 4 * j
                off = 128 * r if r >= 0 else 0
                sc = ps.tile([128, 2 * SW], F32, tag="big", bufs=2, name="sc")
                nc.tensor.matmul(sc[:, off:SW], KT_sb[g0][:, bass.ts(i, 128)],
                                 qtp[:, off:SW], start=True, stop=True)
                nc.tensor.matmul(sc[:, SW + off:2 * SW], KT_sb[g1][:, bass.ts(i, 128)],
                                 qtp[:, SW + off:2 * SW], start=True, stop=True)
                pr = probs_pool.tile([128, 2 * SW], BF16, name="pr")
                nc.scalar.activation(out=pr[:, off:2 * SW], in_=sc[:, off:2 * SW],
                                     func=mybir.ActivationFunctionType.Exp)
                if r >= 0:
                    nc.vector.tensor_mul(pr[:, off:2 * SW], pr[:, off:2 * SW],
                                         masks_sb[:, r, off:2 * SW])
                return pr, off

            def emit_lav(i, pr, off):
                first, last = (i == 0), (i == nlast)
                nc.tensor.matmul(lpack[0:1, off:SW], ones_t, pr[:, off:SW],
                                 start=first, stop=last, skip_group_check=True)
                nc.tensor.matmul(lpack[32:33, off:SW], ones_t, pr[:, SW + off:2 * SW],
                                 start=first, stop=last, skip_group_check=True)
                nc.tensor.matmul(av[:, off:SW], V_sb[g0][:, i, :], pr[:, off:SW],
                                 start=first, stop=last, skip_group_check=True)
                nc.tensor.matmul(av[:, SW + off:2 * SW], V_sb[g1][:, i, :],
                                 pr[:, SW + off:2 * SW],
                                 start=first, stop=last, skip_group_check=True)

            pending = [emit_sc(0)]
            if nlast >= 1:
                pending.append(emit_sc(1))
            for i in range(nlast + 1):
                pr, off = pending.pop(0)
                emit_lav(i, pr, off)
                if i + 2 <= nlast:
                    pending.append(emit_sc(i + 2))

            # normalization for this pair
            l_sb = small_pool.tile([64, SW], F32R, tag="l_sb", name="l_sb")
            nc.scalar.copy(out=l_sb[0:33, :], in_=lpack[0:33, :])
            for a in range(2):
                row = slice(32 * a, 32 * a + 1)
                rb = ps.tile([128, SW], F32, tag="aux", bufs=2, name="rb")
                nc.tensor.matmul(rb, ones_mat[row, :], l_sb[row, :],
                                 start=True, stop=True)
                rbs = small_pool.tile([128, SW], F32, tag="rbs", name="rbs")
                nc.vector.reciprocal_approx_fast(out=rbs, in_=rb)
                ot = out_pool.tile([128, SW], BF16, name="ot")
                nc.vector.tensor_mul(ot, av[:, bass.ts(a, SW)], rbs)
                outTj[2 * P + a] = ot

        # ---- out-projection for window j (d-tile pairs, 6-tile DMA groups) ----
        for q6 in range(4):
            ys6 = ys_pool.tile([128, 6, SW], BF16, name="ys6")
            for dp in range(3):
                dpg = 3 * q6 + dp
                wot = wo_pool.tile([128, 2, QT_PER_CORE, 128], BF16, name="wot")
                nc.gpsimd.dma_start(out=wot, in_=wo2[dpg])
                yp = ps.tile([128, 2 * SW], F32, tag="big", bufs=2, name="yp")
                for h in range(2):
                    hw = bass.ts(h, SW)
                    for u in range(QT_PER_CORE):
                        nc.tensor.matmul(yp[:, hw], wot[:, h, u, :], outTj[u],
                                         start=(u == 0), stop=(u == QT_PER_CORE - 1))
                nc.vector.tensor_copy(
                    out=ys6[:, 2 * dp:2 * dp + 2, :],
                    in_=yp.rearrange("p (h s) -> p h s", h=2))
            nc.sync.dma_start(out=y4[j, q6], in_=ys6)


def build_nc():
    nc = bacc.Bacc("TRN2", target_bir_lowering=False, debug=False, num_devices=8)
    io = {
        "x4": nc.dram_tensor("x4", [NJ, 128, NKT, SW], BF16, kind="ExternalInput"),
        "w10": nc.dram_tensor("w10", [NDT, 128, NKT, 128], BF16, kind="ExternalInput"),
        "wo2": nc.dram_tensor("wo2", [NKT // 2, 128, 2, QT_PER_CORE, 128], BF16,
                              kind="ExternalInput"),
        "ropeC": nc.dram_tensor("ropeC", [HD, S], BF16, kind="ExternalInput"),
        "ropeS": nc.dram_tensor("ropeS", [HD, S], BF16, kind="ExternalInput"),
        "masks": nc.dram_tensor("masks", [128, 4, 2 * SW], BF16, kind="ExternalInput"),
        "swp": nc.dram_tensor("swp", [128, 128], BF16, kind="ExternalInput"),
        "y4": nc.dram_tensor("y4", [NJ, 4, 128, 6, SW], BF16, kind="ExternalOutput"),
    }
    with tile.TileContext(nc) as tc:
        with ExitStack() as ctx:
            _build_body(nc, tc, io, ctx)
    nc.compile()
    return nc


_NC = None


def _get_nc():
    global _NC
    if _NC is None:
        _NC = build_nc()
    return _NC


def make_in_maps(x, wq, wk, wv, wo, freqs_cos, freqs_sin):
    x = np.asarray(x, np.float32)
    wq = np.asarray(wq, np.float32)
    wk = np.asarray(wk, np.float32)
    wv = np.asarray(wv, np.float32)
    wo = np.asarray(wo, np.float32)
    cos = np.asarray(freqs_cos, np.float32)
    sin = np.asarray(freqs_sin, np.float32)

    wq_p = (wq.reshape(DIM, NH, HD)[:, :, _PERM] * SCALE).astype(BF)
    wk_p = wk.reshape(DIM, NKV, HD)[:, :, _PERM].astype(BF)
    wv_r = wv.reshape(DIM, NKV, HD).astype(BF)
    wo_r = wo.reshape(NH, HD, DIM)

    ropeC = np.ascontiguousarray(np.concatenate([cos.T, cos.T], 0)).astype(BF)
    ropeS = np.ascontiguousarray(np.concatenate([-sin.T, sin.T], 0)).astype(BF)

    tt = np.arange(128)[:, None]
    cc = np.arange(2 * SW)[None, :] % SW
    # [128, 4, 2*SW]: masks[:, r, c] is the diag-block pattern for block r,
    # applied to both units' 512-col slices of the paired probs tile
    masks = np.stack([(128 * r + tt <= cc) for r in range(4)], axis=1).astype(BF)

    swp = np.zeros((128, 128), BF)
    swp[np.arange(128), (np.arange(128) + 64) % 128] = 1.0

    in_maps = []
    for c in range(8):
        b, p = divmod(c, 4)
        # per-core weight slices in on-chip tile layout
        wq_c = wq_p[:, 6 * p:6 * p + 6, :]          # [DIM, 6, 128]
        wk_c = wk_p[:, 2 * p:2 * p + 2, :]          # [DIM, 2, 128]
        wv_c = wv_r[:, 2 * p:2 * p + 2, :]          # [DIM, 2, 128]
        # w10[dt] = [128p, 24k, 128d] with DIM rows split as (k, p)
        wcat = np.concatenate([wq_c, wk_c, wv_c], axis=1)   # [DIM, 10, 128]
        w10 = np.ascontiguousarray(
            wcat.reshape(NKT, 128, NDT, HD).transpose(2, 1, 0, 3))
        # wo2[dp] = [128p(dv), 2h, 6u, 128dd]; wo rows are (u, p)
        wo_c = wo_r[6 * p:6 * p + 6]                 # [6, 128, DIM]
        wo4 = np.ascontiguousarray(
            wo_c.reshape(QT_PER_CORE, HD, NKT, 128).transpose(2, 1, 0, 3)).astype(BF)
        wo2 = np.ascontiguousarray(
            wo4.reshape(NKT // 2, 2, 128, QT_PER_CORE, 128).transpose(0, 2, 1, 3, 4))
        # x4[j] = [128p, 24k, 512s]
        xT_b = x[b].T                                 # [DIM, S]
        x4 = np.ascontiguousarray(
            xT_b.reshape(NKT, 128, NJ, SW).transpose(2, 1, 0, 3)).astype(BF)
        in_maps.append({
            "x4": x4,
            "w10": w10,
            "wo2": wo2,
            "ropeC": ropeC,
            "ropeS": ropeS,
            "masks": masks,
            "swp": swp,
        })
    return in_maps


def gather(results):
    # y4 [NJ, 4, 128, 6, SW] bf16 per core: y4[j, q6, p, u, s] holds
    # yT[(6*q6+u)*128 + p, j*SW + s]
    y = np.empty((B, S, DIM), np.float32)
    for b in range(B):
        acc = results[4 * b]["y4"].astype(np.float32)
        for p in range(1, 4):
            acc = acc + results[4 * b + p]["y4"].astype(np.float32)
        # -> [j, s, q6, u, p] -> [S, DIM]
        y[b] = acc.transpose(0, 4, 1, 3, 2).reshape(S, DIM)
    return y


def kernel(x, wq, wk, wv, wo, freqs_cos, freqs_sin, **run_kwargs):
    nc = _get_nc()
    in_maps = make_in_maps(x, wq, wk, wv, wo, freqs_cos, freqs_sin)
    res = run_bass_kernel_spmd(nc, in_maps, core_ids=list(range(8)), **run_kwargs)
    out = gather(res.results)
    if run_kwargs:
        return out, res
    return out


# revision 11
# speedup vs baseline: 1.2020x; 1.0543x over previous
"""GQA attention block (RoPE + causal softmax + out-proj) on 8 TRN2 cores.

Sharding: 8 cores = 2 batches x 4 kv-pairs. Core c handles batch c//4 and
kv heads {2p, 2p+1} (p = c%4), i.e. q heads 6p..6p+5. Each core computes its
partial y^T = wo_slice^T @ attn_out^T; the host sums the 4 partials per batch
(bf16 partials, fp32 host accumulation) and transposes back.

Per-core layout: everything stays feature-major [d, s] so no on-device
transposes of large activations are needed. v2 structure:
  - wq/wk/wv tiles persist in SBUF (loaded once, window 0 streams them in).
  - Projections accumulate two 512-col d-tiles into one 2-bank PSUM tile
    so evacuation copies are batched.
  - Attention runs q-head pairs with a depth-2 software pipeline: both
    units' score tiles land in one [128,1024] PSUM tile, ONE batched exp
    serves both, the two l row-sum matmuls issue back-to-back into
    different PE column groups (concurrent), and diagonal blocks are
    causally trimmed to N=512-128r.
  - Out-projection accumulates d-tile pairs and stages bf16 output in
    6-tile groups so each window needs only 4 output DMA descriptors.
Softmax runs without max-subtraction (scores are O(10), exp is safe in
fp32): probs = exp(scores) * binary causal mask, row sums via col-tiled
ones matmuls into PSUM, 1/l applied after a PE broadcast + DVE reciprocal.
"""

import math
from contextlib import ExitStack

import numpy as np
import ml_dtypes

import concourse.bass as bass
import concourse.mybir as mybir
import concourse.tile as tile
from concourse import bacc
from concourse.bass_utils import run_bass_kernel_spmd
from concourse.masks import make_identity

B, S, DIM = 2, 2048, 3072
NH, NKV, HD = 24, 8, 128
QT_PER_CORE = 6   # q head-tiles per core
KV_PER_CORE = 2   # kv heads per core
NDT = QT_PER_CORE + 2 * KV_PER_CORE  # 10 projection d-tiles
NKT = DIM // 128  # 24 contraction tiles
SW = 512          # s-window (matmul moving free dim)
NJ = S // SW      # 4 windows
NTT = S // 128    # 16 t-tiles
NPAIR = NDT // 2  # 5 projection d-tile pairs
SCALE = 1.0 / math.sqrt(HD)

F32 = mybir.dt.float32
F32R = mybir.dt.float32r
BF16 = mybir.dt.bfloat16
BF = ml_dtypes.bfloat16

_PERM = np.concatenate([np.arange(0, HD, 2), np.arange(1, HD, 2)])


def _build_body(nc, tc, io, ctx):
    x4, w10, wo2 = io["x4"], io["w10"], io["wo2"]
    ropeC, ropeS, masks, swp, y4 = (
        io["ropeC"], io["ropeS"], io["masks"], io["swp"], io["y4"])

    singles = ctx.enter_context(tc.tile_pool(name="singles", bufs=1))
    ps = ctx.enter_context(tc.tile_pool(name="ps", bufs=1, space=bass.MemorySpace.PSUM))
    xt_pool = ctx.enter_context(tc.tile_pool(name="xtp", bufs=2))
    wo_pool = ctx.enter_context(tc.tile_pool(name="wotp", bufs=3))
    raw_pool = ctx.enter_context(tc.tile_pool(name="rawp", bufs=2))
    qT_pool = ctx.enter_context(tc.tile_pool(name="qTp", bufs=4))
    probs_pool = ctx.enter_context(tc.tile_pool(name="prp", bufs=4))
    prsum_pool = ctx.enter_context(tc.tile_pool(name="prsp", bufs=2))
    small_pool = ctx.enter_context(tc.tile_pool(name="smp", bufs=2))
    out_pool = ctx.enter_context(tc.tile_pool(name="otp", bufs=13))
    ys_pool = ctx.enter_context(tc.tile_pool(name="ysp", bufs=2))

    # constants / persistent state (const DMAs ride the gpsimd queue so they
    # don't delay the first x/weight loads on sync)
    ropeC_sb = singles.tile([128, S], BF16, tag="ropeC", name="ropeC_sb")
    ropeS_sb = singles.tile([128, S], BF16, tag="ropeS", name="ropeS_sb")
    masks_sb = singles.tile([128, 128], BF16, tag="masks", name="masks_sb")
    swp_sb = singles.tile([128, 128], BF16, tag="swp", name="swp_sb")
    ident = singles.tile([128, 128], F32, tag="ident", name="ident")
    ones_t = singles.tile([128, 1], BF16, tag="ones_t", name="ones_t")
    ones_mat = singles.tile([128, 128], F32R, tag="ones_mat", name="ones_mat")
    ones_mat0 = singles.tile([128, 128], F32, tag="ones_mat0", name="ones_mat0")
    make_identity(nc, ident)
    nc.vector.memset(ones_t, 1.0)
    nc.vector.memset(ones_mat0, 1.0)
    nc.scalar.copy(out=ones_mat, in_=ones_mat0)

    # persistent weight tiles: w10_sb[dt] = [128p, 24k, 128d], streamed in by
    # window 0 on the gpsimd queue (each queue sprays across all 16 DMA
    # engines, ~350GB/s aggregate), reused by windows 1-3. Priority order:
    # first projection pair's weights, then the rope/swap consts (needed by
    # the first post at ~14us), then the remaining pairs.
    w10_sb = [singles.tile([128, NKT, 128], BF16, tag=f"w{dt}", name=f"w10_{dt}")
              for dt in range(NDT)]
    KT_sb = [singles.tile([128, S], BF16, tag=f"KT{g}", name=f"KT{g}")
             for g in range(KV_PER_CORE)]
    V_sb = [singles.tile([128, NTT, 128], BF16, tag=f"V{g}", name=f"V{g}")
            for g in range(KV_PER_CORE)]

    for dt in range(2):
        nc.gpsimd.dma_start(out=w10_sb[dt], in_=w10[dt])
    nc.gpsimd.dma_start(out=ropeC_sb, in_=ropeC[:])
    nc.gpsimd.dma_start(out=ropeS_sb, in_=ropeS[:])
    nc.gpsimd.dma_start(out=swp_sb, in_=swp[:])
    nc.gpsimd.dma_start(out=masks_sb, in_=masks[:])
    for dt in range(2, NDT):
        nc.gpsimd.dma_start(out=w10_sb[dt], in_=w10[dt])

    def emit_outproj_chunk(jm1, outT, chunk):
        # two 6-d-tile output groups of window jm1's out-projection; emitted
        # inside window jm1+1's attention so the PE chews these matmuls while
        # the softmax-normalization chains of a pair drain.
        for q6 in (2 * chunk, 2 * chunk + 1):
            ys6 = ys_pool.tile([128, 6, SW], BF16, name="ys6")
            for dp in range(3):
                dpg = 3 * q6 + dp
                wot = wo_pool.tile([128, 2, QT_PER_CORE, 128], BF16, name="wot")
                nc.gpsimd.dma_start(out=wot, in_=wo2[dpg])
                yp = ps.tile([128, 2 * SW], F32, tag="big", bufs=2, name="yp")
                for h in range(2):
                    hw = bass.ts(h, SW)
                    for u in range(QT_PER_CORE):
                        nc.tensor.matmul(yp[:, hw], wot[:, h, u, :], outT[u],
                                         start=(u == 0), stop=(u == QT_PER_CORE - 1))
                nc.vector.tensor_copy(
                    out=ys6[:, 2 * dp:2 * dp + 2, :],
                    in_=yp.rearrange("p (h s) -> p h s", h=2))
            nc.sync.dma_start(out=y4[jm1, q6], in_=ys6)

    prev_outT = None
    for j in range(NJ):
        jw = bass.ts(j, SW)

        # x chunk: fine slices up front so the first matmuls start early,
        # alternating the sync/scalar queues
        xt = xt_pool.tile([128, NKT, SW], BF16, name="xt")
        xsl = [(0, 1), (1, 2), (2, 4), (4, 6), (6, 9), (9, 12),
               (12, 16), (16, 20), (20, 24)]
        for n, (k0, k1) in enumerate(xsl):
            eng = nc.sync if n % 2 == 0 else nc.scalar
            eng.dma_start(out=xt[:, k0:k1, :], in_=x4[j, :, k0:k1, :])

        # ---- projections for window j (5 d-tile pairs) ----
        # pairs 0-2 -> q tiles, 3 -> k heads, 4 -> v heads. Post-processing
        # (swap matmul / V transposes + rope) for pair p is deferred until
        # after pair p+1's accumulation so the PE never waits on the PSUM
        # evacuation.
        qTj = [None] * (QT_PER_CORE // 2)  # q pair tiles [128, 1024]
        post = []

        def _post_qk(p, raw, sw_ps):
            def run():
                for h in range(2):
                    dt = 2 * p + h
                    hw = bass.ts(h, SW)
                    if dt < 6:
                        if h == 0:
                            qTj[p] = qT_pool.tile([128, 2 * SW], BF16, name="qt")
                        dest = qTj[p][:, hw]
                    else:
                        dest = KT_sb[dt - 6][:, jw]
                    nc.vector.tensor_mul(dest, raw[:, hw], ropeC_sb[:, jw])
                    t2 = raw_pool.tile([128, SW], BF16, tag="t2", bufs=2, name="t2")
                    nc.vector.tensor_mul(t2, sw_ps[:, hw], ropeS_sb[:, jw])
                    nc.vector.tensor_add(dest, dest, t2)
            return run

        def _post_v(vraw):
            def run():
                tp = ps.tile([128, 2 * SW], F32, tag="av", bufs=1, name="tp")
                for g in range(2):
                    for rr in range(4):
                        c = g * 4 + rr
                        nc.tensor.transpose(tp[:, bass.ts(c, 128)],
                                            vraw[:, bass.ts(c, 128)], ident)
                for g in range(2):
                    nc.scalar.copy(
                        out=V_sb[g][:, 4 * j:4 * j + 4, :],
                        in_=tp[:, bass.ts(g, SW)].rearrange("p (r t) -> p r t", r=4))
            return run

        for p in range(NPAIR):
            kind = "q" if p < 3 else ("k" if p == 3 else "v")
            pp = ps.tile([128, 2 * SW], F32, tag="big", bufs=2, name="pp")
            for h in range(2):
                dt = 2 * p + h
                hw = bass.ts(h, SW)
                for k in range(NKT):
                    nc.tensor.matmul(pp[:, hw], w10_sb[dt][:, k, :], xt[:, k, :],
                                     start=(k == 0), stop=(k == NKT - 1))
            if kind == "v":
                vraw = raw_pool.tile([128, 2 * SW], F32, tag="raw", name="vraw")
                nc.any.tensor_copy(out=vraw, in_=pp)
                post.append(_post_v(vraw))
            else:
                raw = raw_pool.tile([128, 2 * SW], BF16, tag="raw", name="raw")
                nc.any.tensor_copy(out=raw, in_=pp)
                sw_ps = ps.tile([128, 2 * SW], F32, tag="av", bufs=1, name="sw_ps")
                for h in range(2):
                    hw = bass.ts(h, SW)
                    nc.tensor.matmul(sw_ps[:, hw], swp_sb, raw[:, hw],
                                     start=True, stop=True)
                post.append(_post_qk(p, raw, sw_ps))
            if len(post) > 1:
                post.pop(0)()
        post.pop(0)()

        # ---- attention for window j: 3 unit-pairs, depth-2 pipeline ----
        outTj = [None] * QT_PER_CORE
        nlast = 4 * j + 3
        for P in range(3):
            u0, u1 = 2 * P, 2 * P + 1
            g0, g1 = u0 // 3, u1 // 3
            qtp = qTj[P]
            lpack = ps.tile([128, SW], F32, tag="aux", bufs=2, name="lpack")
            av = ps.tile([128, 2 * SW], F32, tag="av", bufs=1, name="av")

            def emit_sc(i):
                r = i - 4 * j
                off = 128 * r if r >= 0 else 0
                sc = ps.tile([128, 2 * SW], F32, tag="big", bufs=2, name="sc")
                nc.tensor.matmul(sc[:, off:SW], KT_sb[g0][:, bass.ts(i, 128)],
                                 qtp[:, off:SW], start=True, stop=True)
                nc.tensor.matmul(sc[:, SW + off:2 * SW], KT_sb[g1][:, bass.ts(i, 128)],
                                 qtp[:, SW + off:2 * SW], start=True, stop=True)
                pr = probs_pool.tile([128, 2 * SW], BF16, name="pr")
                nc.scalar.activation(out=pr[:, off:2 * SW], in_=sc[:, off:2 * SW],
                                     func=mybir.ActivationFunctionType.Exp)
                if r >= 0:
                    # only the 128-wide diagonal block needs masking
                    nc.vector.tensor_mul(pr[:, off:off + 128], pr[:, off:off + 128],
                                         masks_sb)
                    nc.vector.tensor_mul(pr[:, SW + off:SW + off + 128],
                                         pr[:, SW + off:SW + off + 128], masks_sb)
                return pr, off

            def emit_l(rhs0, rhs1, first, last):
                nc.tensor.matmul(lpack[0:1, :], ones_t, rhs0,
                                 start=first, stop=last, skip_group_check=True)
                nc.tensor.matmul(lpack[32:33, :], ones_t, rhs1,
                                 start=first, stop=last, skip_group_check=True)

            def emit_lav(i, pr, off, prev_pr):
                first, last = (i == 0), (i == nlast)
                if off == 0 and i < 4 * j:
                    # non-diagonal: fold pairs of probs tiles on the DVE and
                    # run the l row-sum matmuls at half rate
                    if i % 2 == 1:
                        prs = prsum_pool.tile([128, 2 * SW], BF16, name="prs")
                        nc.vector.tensor_add(prs, prev_pr, pr)
                        emit_l(prs[:, 0:SW], prs[:, SW:2 * SW], i == 1, False)
                else:
                    nc.tensor.matmul(lpack[0:1, off:SW], ones_t, pr[:, off:SW],
                                     start=first, stop=last, skip_group_check=True)
                    nc.tensor.matmul(lpack[32:33, off:SW], ones_t,
                                     pr[:, SW + off:2 * SW],
                                     start=first, stop=last, skip_group_check=True)
                nc.tensor.matmul(av[:, off:SW], V_sb[g0][:, i, :], pr[:, off:SW],
                                 start=first, stop=last, skip_group_check=True)
                nc.tensor.matmul(av[:, SW + off:2 * SW], V_sb[g1][:, i, :],
                                 pr[:, SW + off:2 * SW],
                                 start=first, stop=last, skip_group_check=True)

            pending = [emit_sc(0)]
            if nlast >= 1:
                pending.append(emit_sc(1))
            prev_pr = None
            for i in range(nlast + 1):
                pr, off = pending.pop(0)
                emit_lav(i, pr, off, prev_pr)
                prev_pr = pr
                if i + 2 <= nlast:
                    pending.append(emit_sc(i + 2))

            # normalization for this pair
            l_sb = small_pool.tile([64, SW], F32R, tag="l_sb", name="l_sb")
            nc.scalar.copy(out=l_sb[0:33, :], in_=lpack[0:33, :])
            for a in range(2):
                row = slice(32 * a, 32 * a + 1)
                rb = ps.tile([128, SW], F32, tag="aux", bufs=2, name="rb")
                nc.tensor.matmul(rb, ones_mat[row, :], l_sb[row, :],
                                 start=True, stop=True)
                rbs = small_pool.tile([128, SW], F32, tag="rbs", name="rbs")
                nc.vector.reciprocal_approx_fast(out=rbs, in_=rb)
                ot = out_pool.tile([128, SW], BF16, name="ot")
                nc.vector.tensor_mul(ot, av[:, bass.ts(a, SW)], rbs)
                outTj[2 * P + a] = ot

            # previous window's out-projection fills the pair-boundary bubble
            if prev_outT is not None and P < 2:
                emit_outproj_chunk(j - 1, prev_outT, P)

        prev_outT = outTj

    # final window's out-projection (nothing left to interleave it with)
    emit_outproj_chunk(NJ - 1, prev_outT, 0)
    emit_outproj_chunk(NJ - 1, prev_outT, 1)


def build_nc():
    nc = bacc.Bacc("TRN2", target_bir_lowering=False, debug=False, num_devices=8)
    io = {
        "x4": nc.dram_tensor("x4", [NJ, 128, NKT, SW], BF16, kind="ExternalInput"),
        "w10": nc.dram_tensor("w10", [NDT, 128, NKT, 128], BF16, kind="ExternalInput"),
        "wo2": nc.dram_tensor("wo2", [NKT // 2, 128, 2, QT_PER_CORE, 128], BF16,
                              kind="ExternalInput"),
        "ropeC": nc.dram_tensor("ropeC", [HD, S], BF16, kind="ExternalInput"),
        "ropeS": nc.dram_tensor("ropeS", [HD, S], BF16, kind="ExternalInput"),
        "masks": nc.dram_tensor("masks", [128, 128], BF16, kind="ExternalInput"),
        "swp": nc.dram_tensor("swp", [128, 128], BF16, kind="ExternalInput"),
        "y4": nc.dram_tensor("y4", [NJ, 4, 128, 6, SW], BF16, kind="ExternalOutput"),
    }
    with tile.TileContext(nc) as tc:
        with ExitStack() as ctx:
            _build_body(nc, tc, io, ctx)
    nc.compile()
    return nc


_NC = None


def _get_nc():
    global _NC
    if _NC is None:
        _NC = build_nc()
    return _NC


def make_in_maps(x, wq, wk, wv, wo, freqs_cos, freqs_sin):
    x = np.asarray(x, np.float32)
    wq = np.asarray(wq, np.float32)
    wk = np.asarray(wk, np.float32)
    wv = np.asarray(wv, np.float32)
    wo = np.asarray(wo, np.float32)
    cos = np.asarray(freqs_cos, np.float32)
    sin = np.asarray(freqs_sin, np.float32)

    wq_p = (wq.reshape(DIM, NH, HD)[:, :, _PERM] * SCALE).astype(BF)
    wk_p = wk.reshape(DIM, NKV, HD)[:, :, _PERM].astype(BF)
    wv_r = wv.reshape(DIM, NKV, HD).astype(BF)
    wo_r = wo.reshape(NH, HD, DIM)

    ropeC = np.ascontiguousarray(np.concatenate([cos.T, cos.T], 0)).astype(BF)
    ropeS = np.ascontiguousarray(np.concatenate([-sin.T, sin.T], 0)).astype(BF)

    tt = np.arange(128)[:, None]
    cc = np.arange(128)[None, :]
    # [128, 128] lower-triangle mask for the diagonal 128-block of any r
    masks = (tt <= cc).astype(BF)

    swp = np.zeros((128, 128), BF)
    swp[np.arange(128), (np.arange(128) + 64) % 128] = 1.0

    in_maps = []
    for c in range(8):
        b, p = divmod(c, 4)
        # per-core weight slices in on-chip tile layout
        wq_c = wq_p[:, 6 * p:6 * p + 6, :]          # [DIM, 6, 128]
        wk_c = wk_p[:, 2 * p:2 * p + 2, :]          # [DIM, 2, 128]
        wv_c = wv_r[:, 2 * p:2 * p + 2, :]          # [DIM, 2, 128]
        # w10[dt] = [128p, 24k, 128d] with DIM rows split as (k, p)
        wcat = np.concatenate([wq_c, wk_c, wv_c], axis=1)   # [DIM, 10, 128]
        w10 = np.ascontiguousarray(
            wcat.reshape(NKT, 128, NDT, HD).transpose(2, 1, 0, 3))
        # wo2[dp] = [128p(dv), 2h, 6u, 128dd]; wo rows are (u, p)
        wo_c = wo_r[6 * p:6 * p + 6]                 # [6, 128, DIM]
        wo4 = np.ascontiguousarray(
            wo_c.reshape(QT_PER_CORE, HD, NKT, 128).transpose(2, 1, 0, 3)).astype(BF)
        wo2 = np.ascontiguousarray(
            wo4.reshape(NKT // 2, 2, 128, QT_PER_CORE, 128).transpose(0, 2, 1, 3, 4))
        # x4[j] = [128p, 24k, 512s]
        xT_b = x[b].T                                 # [DIM, S]
        x4 = np.ascontiguousarray(
            xT_b.reshape(NKT, 128, NJ, SW).transpose(2, 1, 0, 3)).astype(BF)
        in_maps.append({
            "x4": x4,
            "w10": w10,
            "wo2": wo2,
            "ropeC": ropeC,
            "ropeS": ropeS,
            "masks": masks,
            "swp": swp,
        })
    return in_maps


def gather(results):
    # y4 [NJ, 4, 128, 6, SW] bf16 per core: y4[j, q6, p, u, s] holds
    # yT[(6*q6+u)*128 + p, j*SW + s]
    y = np.empty((B, S, DIM), np.float32)
    for b in range(B):
        acc = results[4 * b]["y4"].astype(np.float32)
        for p in range(1, 4):
            acc = acc + results[4 * b + p]["y4"].astype(np.float32)
        # -> [j, s, q6, u, p] -> [S, DIM]
        y[b] = acc.transpose(0, 4, 1, 3, 2).reshape(S, DIM)
    return y


def kernel(x, wq, wk, wv, wo, freqs_cos, freqs_sin, **run_kwargs):
    nc = _get_nc()
    in_maps = make_in_maps(x, wq, wk, wv, wo, freqs_cos, freqs_sin)
    res = run_bass_kernel_spmd(nc, in_maps, core_ids=list(range(8)), **run_kwargs)
    out = gather(res.results)
    if run_kwargs:
        return out, res
    return out
